# revision 15
# baseline (speedup 1.0000x reference)
"""Trainium2 Bass kernel for ContinuousIntegratedKoopmanOperator.

reference: odeint(dz/dt = z @ W) sampled at t = DT*[1..T], y0 = x at t[0].
Closed form (time-invariant linear ODE): out[:, j, :] = x @ expm(DT*j*W).

Strategy (v4 -- fp16 end-to-end, store-roofline bound, tuned head):
  host: Mj = expm(DT*j*W) table in float64 -> fp16; x shard transposed
        -> fp16. Tolerance is 2e-2; fp16 rounding contributes ~4e-4.
  device (8 cores, batch-sharded 1024 rows each):
        out_tile = x_tile @ M_block, ONE fp16 matmul per 512-col block.
        PE pre-warmed with dummy matmuls during the NEFF preamble + load
        window so HAM is un-throttled when real work starts.
        PSUM rotated as 4 pairs of 2 banks (deep pipeline: drains trail
        matmuls without ever stalling them -- v3's 2-deep half scheme
        serialized drain->matmul->drain and lost 30%).
        Drains (f32->fp16) 1024 cols, even pairs on Vector, odd on
        Scalar, into 8 dedicated per-tile staging buffers (producer
        runs ahead; stores stream at pure bandwidth).
        Stores on sync ring: tile 0 in 4 x 512KB units (early stream
        start), tiles 1-7 as 2MB tiles. Loads: x + M units 0,1 on the
        sync ring (fast start), M units 2,3 on the scalar ring.
  sync: raw bass, explicit sems, per-engine drain sems; matmul sem
        incremented once per pair.
"""
import numpy as np

DT = 0.01
B, D, T = 8192, 128, 64
NCORES = 8
BSH = B // NCORES          # 1024 rows per core
NTILES = BSH // 128        # 8 batch tiles per core
BW = 512                   # j-block width (4 j's of 128)
NBLK = (T * D) // BW       # 16 blocks per tile
NPAIR = NBLK // 2          # 8 pairs per tile (drain unit = 2 banks)
UW = 2048                  # store-unit width for tile 0
NUNIT = (T * D) // UW      # 4 store units for tile 0
MCW = 1024                 # M load-chunk width (256KB chunks)
NMC = (T * D) // MCW       # 8 M chunks
DUMMY = 9                  # PE warmup matmuls (end just after M chunk 0 lands)

_CACHE = {}


def _expm_table(W: np.ndarray) -> np.ndarray:
    """(D, T*D) float64: columns [j*D:(j+1)*D] = expm(DT*j*W)."""
    A = DT * W.astype(np.float64)
    M1 = np.eye(D, dtype=np.float64)
    term = np.eye(D, dtype=np.float64)
    for n in range(1, 24):
        term = term @ A / n
        M1 += term
    Ms = np.empty((T, D, D), dtype=np.float64)
    Ms[0] = np.eye(D)
    for j in range(1, T):
        Ms[j] = Ms[j - 1] @ M1
    return np.ascontiguousarray(Ms.transpose(1, 0, 2).reshape(D, T * D))


def _build_nc():
    import concourse.bass as bass
    import concourse.mybir as mybir

    f32 = mybir.dt.float32
    f16 = mybir.dt.float16

    nc = bass.Bass(trn_type="TRN2")
    xT_d = nc.dram_tensor("xT", (D, NTILES * 128), f16, kind="ExternalInput")
    M_d = nc.dram_tensor("M", (D, T * D), f16, kind="ExternalInput")
    out_d = nc.dram_tensor("out", (BSH, T * D), f16, kind="ExternalOutput")

    xT_s = nc.alloc_sbuf_tensor("xT_s", [D, NTILES * 128], f16)
    M_s = nc.alloc_sbuf_tensor("M_s", [D, T * D], f16)
    stg = [nc.alloc_sbuf_tensor(f"stg{i}", [128, T * D], f16) for i in range(NTILES)]
    psum = nc.alloc_psum_tensor("acc", [128, 8 * 512], f32)  # 4 pairs of 2 banks

    s_ldx0 = nc.alloc_semaphore("s_ldx0")   # x tile-0 slice
    s_ldxr = nc.alloc_semaphore("s_ldxr")   # x tiles 1-7
    s_ldm = [nc.alloc_semaphore(f"s_ldm{c}") for c in range(NMC)]
    s_mm = nc.alloc_semaphore("s_mm")      # +1 per completed pair
    s_dv = nc.alloc_semaphore("s_dv")      # Vector drains (even pairs)
    s_da = nc.alloc_semaphore("s_da")      # Scalar drains (odd pairs)
    s_out = nc.alloc_semaphore("s_out")    # store completions
    s_boot = nc.alloc_semaphore("s_boot")

    all_sems = [s_ldx0, s_ldxr, *s_ldm, s_mm, s_dv, s_da, s_out, s_boot]
    nums = sorted(s.num for s in all_sems)
    assert nums == list(range(nums[0], nums[-1] + 1)), "sems not contiguous"
    sem_range = range(nums[0], nums[-1] + 1)

    nc.gpsimd.dma_reset(sem_range)

    # drain bookkeeping: even pair -> Vector, odd -> Scalar.
    # count on that engine's sem after pair (i, q) drained:
    def dr_val(i, q):
        return NPAIR // 2 * i + q // 2 + 1

    def drain_done_wait(eng, i, q):
        eng.wait_ge(s_dv if q % 2 == 0 else s_da, dr_val(i, q))

    n_stores = NUNIT + (NTILES - 1)

    with nc.Block() as block:
        @block.sync
        def _(sync):
            sync.sem_clear(sem_range)
            sync.nop().then_inc(s_boot, 1)
            # loads: x tile-0 slice + even M chunks here (odds on scalar)
            sync.dma_start(out=xT_s[:, 0:128], in_=xT_d[:, 0:128]).then_inc(s_ldx0, 16)
            for c in range(0, NMC, 2):
                sync.dma_start(out=M_s[:, c * MCW:(c + 1) * MCW],
                               in_=M_d[:, c * MCW:(c + 1) * MCW]).then_inc(s_ldm[c], 16)
            # stores: tile 0 per 2048-col unit (early start), tiles 1-7 full
            for u in range(NUNIT):
                sync.wait_ge(s_dv, u + 1)      # even pairs 0..2u drained
                sync.wait_ge(s_da, u + 1)      # odd pairs 1..2u+1 drained
                sync.dma_start(out=out_d[0:128, u * UW:(u + 1) * UW],
                               in_=stg[0][:, u * UW:(u + 1) * UW]).then_inc(s_out, 16)
            for i in range(1, NTILES):
                sync.wait_ge(s_dv, NPAIR // 2 * (i + 1))
                sync.wait_ge(s_da, NPAIR // 2 * (i + 1))
                sync.dma_start(out=out_d[i * 128:(i + 1) * 128, :],
                               in_=stg[i][:, :]).then_inc(s_out, 16)
            sync.wait_ge(s_out, 16 * n_stores)

        @block.scalar
        def _(scalar):
            scalar.wait_ge(s_boot, 1)
            for c in range(1, NMC, 2):
                scalar.dma_start(out=M_s[:, c * MCW:(c + 1) * MCW],
                                 in_=M_d[:, c * MCW:(c + 1) * MCW]).then_inc(s_ldm[c], 16)
            scalar.dma_start(out=xT_s[:, 128:], in_=xT_d[:, 128:]).then_inc(s_ldxr, 16)
            # odd-pair drains: f32 PSUM -> fp16 staging
            for i in range(NTILES):
                for q in range(1, NPAIR, 2):
                    P = i * NPAIR + q
                    scalar.wait_ge(s_mm, i * NPAIR + q + 1)
                    pp = (P % 4) * 1024
                    scalar.copy(out=stg[i][:, q * 1024:(q + 1) * 1024],
                                in_=psum[:, pp:pp + 1024]).then_inc(s_da, 1)

        @block.vector
        def _(vector):
            vector.wait_ge(s_boot, 1)
            for i in range(NTILES):
                for q in range(0, NPAIR, 2):
                    P = i * NPAIR + q
                    vector.wait_ge(s_mm, i * NPAIR + q + 1)
                    pp = (P % 4) * 1024
                    vector.tensor_copy(out=stg[i][:, q * 1024:(q + 1) * 1024],
                                       in_=psum[:, pp:pp + 1024]).then_inc(s_dv, 1)

        @block.tensor
        def _(tensor):
            # PE warmup: un-gated dummy matmuls keep the PE busy through
            # the NEFF preamble + load window so HAM is at K=8/8 when real
            # matmuls start. Outputs are garbage, overwritten (start=True)
            # before any s_mm-gated drain can read them.
            for k in range(DUMMY):
                pb = (k % 8) * 512
                tensor.matmul(psum[:, pb:pb + 512], xT_s[:, 0:128],
                              M_s[:, 0:512], start=True, stop=True)
            tensor.wait_ge(s_boot, 1)
            tensor.wait_ge(s_ldx0, 16)
            for i in range(NTILES):
                for b in range(NBLK):
                    q = b // 2
                    P = i * NPAIR + q
                    if i == 1 and b == 0:
                        tensor.wait_ge(s_ldxr, 16)
                    if i == 0 and b % 2 == 0:
                        tensor.wait_ge(s_ldm[q], 16)  # chunk q = pair q cols
                    if b % 2 == 0 and P >= 4:   # pair slot reused: drain done?
                        i_, q_ = divmod(P - 4, NPAIR)
                        drain_done_wait(tensor, i_, q_)
                    pb = (P % 4) * 1024 + (b % 2) * 512
                    mm = tensor.matmul(psum[:, pb:pb + 512],
                                       xT_s[:, i * 128:(i + 1) * 128],
                                       M_s[:, b * BW:(b + 1) * BW],
                                       start=True, stop=True)
                    if b % 2 == 1:
                        mm.then_inc(s_mm, 1)

    return nc


def _prep_inputs(x: np.ndarray, Mcat64: np.ndarray):
    """Per-core input maps from the (D, T*D) float64 expm table."""
    Mb = Mcat64.astype(np.float16)
    maps = []
    for c in range(NCORES):
        xT = np.ascontiguousarray(x[c * BSH:(c + 1) * BSH].T).astype(np.float16)
        maps.append({"xT": xT, "M": Mb})
    return maps


def run_on_device(x: np.ndarray, Mcat64: np.ndarray, trace: bool = False):
    from concourse.bass_utils import run_bass_kernel_spmd

    if "nc" not in _CACHE:
        _CACHE["nc"] = _build_nc()
    nc = _CACHE["nc"]

    in_maps = _prep_inputs(x, Mcat64)
    res = run_bass_kernel_spmd(nc, in_maps, core_ids=list(range(NCORES)), trace=trace)
    out = np.empty((B, T, D), dtype=np.float32)
    for c in range(NCORES):
        out[c * BSH:(c + 1) * BSH] = res.results[c]["out"].astype(np.float32).reshape(BSH, T, D)
    return out, res


def kernel(x, W, T):
    x = np.asarray(x, dtype=np.float32)
    W = np.asarray(W, dtype=np.float32)
    assert int(T) == 64 and x.shape == (B, D) and W.shape == (D, D)
    Mcat64 = _expm_table(W)
    out, _ = run_on_device(x, Mcat64, trace=False)
    return out


# revision 16
# speedup vs baseline: 1.1205x; 1.1205x over previous
"""Trainium2 Bass kernel for ContinuousIntegratedKoopmanOperator.

reference: odeint(dz/dt = z @ W) sampled at t = DT*[1..T], y0 = x at t[0].
Closed form (time-invariant linear ODE): out[:, j, :] = x @ expm(DT*j*W).

Strategy (v4 -- fp16 end-to-end, store-roofline bound, tuned head):
  host: Mj = expm(DT*j*W) table in float64 -> fp16; x shard transposed
        -> fp16. Tolerance is 2e-2; fp16 rounding contributes ~4e-4.
  device (8 cores, batch-sharded 1024 rows each):
        out_tile = x_tile @ M_block, ONE fp16 matmul per 512-col block.
        PE pre-warmed with dummy matmuls during the NEFF preamble + load
        window so HAM is un-throttled when real work starts.
        PSUM rotated as 4 pairs of 2 banks (deep pipeline: drains trail
        matmuls without ever stalling them -- v3's 2-deep half scheme
        serialized drain->matmul->drain and lost 30%).
        Drains (f32->fp16) 1024 cols, even pairs on Vector, odd on
        Scalar, into 8 dedicated per-tile staging buffers (producer
        runs ahead; stores stream at pure bandwidth).
        Stores on sync ring: tile 0 in 4 x 512KB units (early stream
        start), tiles 1-7 as 2MB tiles. Loads: x + M units 0,1 on the
        sync ring (fast start), M units 2,3 on the scalar ring.
  sync: raw bass, explicit sems, per-engine drain sems; matmul sem
        incremented once per pair.
"""
import numpy as np

DT = 0.01
B, D, T = 8192, 128, 64
NCORES = 8
BSH = B // NCORES          # 1024 rows per core
NTILES = BSH // 128        # 8 batch tiles per core
BW = 512                   # j-block width (4 j's of 128)
NBLK = (T * D) // BW       # 16 blocks per tile
NPAIR = NBLK // 2          # 8 pairs per tile (drain unit = 2 banks)
UW = 2048                  # store-unit width for tile 0
NUNIT = (T * D) // UW      # 4 store units for tile 0
MCW = 1024                 # M load-chunk width (256KB chunks)
NMC = (T * D) // MCW       # 8 M chunks
DUMMY = 17                 # PE warmup matmuls (end just after M chunk 0 lands)

_CACHE = {}


def _expm_table(W: np.ndarray) -> np.ndarray:
    """(D, T*D) float64: columns [j*D:(j+1)*D] = expm(DT*j*W)."""
    A = DT * W.astype(np.float64)
    M1 = np.eye(D, dtype=np.float64)
    term = np.eye(D, dtype=np.float64)
    for n in range(1, 24):
        term = term @ A / n
        M1 += term
    Ms = np.empty((T, D, D), dtype=np.float64)
    Ms[0] = np.eye(D)
    for j in range(1, T):
        Ms[j] = Ms[j - 1] @ M1
    return np.ascontiguousarray(Ms.transpose(1, 0, 2).reshape(D, T * D))


def _build_nc():
    import concourse.bass as bass
    import concourse.mybir as mybir

    f32 = mybir.dt.float32
    f16 = mybir.dt.float16

    nc = bass.Bass(trn_type="TRN2")
    xT_d = nc.dram_tensor("xT", (D, NTILES * 128), f16, kind="ExternalInput")
    M_d = nc.dram_tensor("M", (D, T * D), f16, kind="ExternalInput")
    out_d = nc.dram_tensor("out", (BSH, T * D), f16, kind="ExternalOutput")

    xT_s = nc.alloc_sbuf_tensor("xT_s", [D, NTILES * 128], f16)
    M_s = nc.alloc_sbuf_tensor("M_s", [D, T * D], f16)
    stg = [nc.alloc_sbuf_tensor(f"stg{i}", [128, T * D], f16) for i in range(NTILES)]
    psum = nc.alloc_psum_tensor("acc", [128, 8 * 512], f32)  # 4 pairs of 2 banks

    s_ldx = nc.alloc_semaphore("s_ldx")
    s_ldm = [nc.alloc_semaphore(f"s_ldm{c}") for c in range(NMC)]
    s_mm = nc.alloc_semaphore("s_mm")      # +1 per completed pair
    s_dv = nc.alloc_semaphore("s_dv")      # Vector drains (even pairs)
    s_da = nc.alloc_semaphore("s_da")      # Scalar drains (odd pairs)
    s_out = nc.alloc_semaphore("s_out")    # store completions
    s_boot = nc.alloc_semaphore("s_boot")

    all_sems = [s_ldx, *s_ldm, s_mm, s_dv, s_da, s_out, s_boot]
    nums = sorted(s.num for s in all_sems)
    assert nums == list(range(nums[0], nums[-1] + 1)), "sems not contiguous"
    sem_range = range(nums[0], nums[-1] + 1)

    nc.gpsimd.dma_reset(sem_range)

    # drain bookkeeping: even pair -> Vector, odd -> Scalar.
    # count on that engine's sem after pair (i, q) drained:
    def dr_val(i, q):
        return NPAIR // 2 * i + q // 2 + 1

    def drain_done_wait(eng, i, q):
        eng.wait_ge(s_dv if q % 2 == 0 else s_da, dr_val(i, q))

    n_stores = NUNIT + (NTILES - 1)

    with nc.Block() as block:
        @block.sync
        def _(sync):
            sync.sem_clear(sem_range)
            sync.nop().then_inc(s_boot, 1)
            # loads: x + even M chunks on this ring (odd chunks on scalar)
            sync.dma_start(out=xT_s[:, :], in_=xT_d[:, :]).then_inc(s_ldx, 16)
            for c in range(0, NMC, 2):
                sync.dma_start(out=M_s[:, c * MCW:(c + 1) * MCW],
                               in_=M_d[:, c * MCW:(c + 1) * MCW]).then_inc(s_ldm[c], 16)
            # stores: tile 0 per 2048-col unit (early start), tiles 1-7 full
            for u in range(NUNIT):
                sync.wait_ge(s_dv, u + 1)      # even pairs 0..2u drained
                sync.wait_ge(s_da, u + 1)      # odd pairs 1..2u+1 drained
                sync.dma_start(out=out_d[0:128, u * UW:(u + 1) * UW],
                               in_=stg[0][:, u * UW:(u + 1) * UW]).then_inc(s_out, 16)
            for i in range(1, NTILES):
                sync.wait_ge(s_dv, NPAIR // 2 * (i + 1))
                sync.wait_ge(s_da, NPAIR // 2 * (i + 1))
                sync.dma_start(out=out_d[i * 128:(i + 1) * 128, :],
                               in_=stg[i][:, :]).then_inc(s_out, 16)
            sync.wait_ge(s_out, 16 * n_stores)

        @block.scalar
        def _(scalar):
            scalar.wait_ge(s_boot, 1)
            for c in range(1, NMC, 2):
                scalar.dma_start(out=M_s[:, c * MCW:(c + 1) * MCW],
                                 in_=M_d[:, c * MCW:(c + 1) * MCW]).then_inc(s_ldm[c], 16)
            # odd-pair drains: f32 PSUM -> fp16 staging
            for i in range(NTILES):
                for q in range(1, NPAIR, 2):
                    P = i * NPAIR + q
                    scalar.wait_ge(s_mm, i * NPAIR + q + 1)
                    pp = (P % 4) * 1024
                    scalar.copy(out=stg[i][:, q * 1024:(q + 1) * 1024],
                                in_=psum[:, pp:pp + 1024]).then_inc(s_da, 1)

        @block.vector
        def _(vector):
            vector.wait_ge(s_boot, 1)
            for i in range(NTILES):
                for q in range(0, NPAIR, 2):
                    P = i * NPAIR + q
                    vector.wait_ge(s_mm, i * NPAIR + q + 1)
                    pp = (P % 4) * 1024
                    vector.tensor_copy(out=stg[i][:, q * 1024:(q + 1) * 1024],
                                       in_=psum[:, pp:pp + 1024]).then_inc(s_dv, 1)

        @block.tensor
        def _(tensor):
            # PE warmup: un-gated dummy matmuls keep the PE busy through
            # the NEFF preamble + load window so HAM is at K=8/8 when real
            # matmuls start. Outputs are garbage, overwritten (start=True)
            # before any s_mm-gated drain can read them.
            for k in range(DUMMY):
                pb = (k % 8) * 512
                tensor.matmul(psum[:, pb:pb + 512], xT_s[:, 0:128],
                              M_s[:, 0:512], start=True, stop=True)
            tensor.wait_ge(s_boot, 1)
            tensor.wait_ge(s_ldx, 16)
            for i in range(NTILES):
                for b in range(NBLK):
                    q = b // 2
                    P = i * NPAIR + q
                    if i == 0 and b % 2 == 0:
                        tensor.wait_ge(s_ldm[q], 16)  # chunk q = pair q cols
                    if b % 2 == 0 and P >= 4:   # pair slot reused: drain done?
                        i_, q_ = divmod(P - 4, NPAIR)
                        drain_done_wait(tensor, i_, q_)
                    pb = (P % 4) * 1024 + (b % 2) * 512
                    mm = tensor.matmul(psum[:, pb:pb + 512],
                                       xT_s[:, i * 128:(i + 1) * 128],
                                       M_s[:, b * BW:(b + 1) * BW],
                                       start=True, stop=True)
                    if b % 2 == 1:
                        mm.then_inc(s_mm, 1)

    return nc


def _prep_inputs(x: np.ndarray, Mcat64: np.ndarray):
    """Per-core input maps from the (D, T*D) float64 expm table."""
    Mb = Mcat64.astype(np.float16)
    maps = []
    for c in range(NCORES):
        xT = np.ascontiguousarray(x[c * BSH:(c + 1) * BSH].T).astype(np.float16)
        maps.append({"xT": xT, "M": Mb})
    return maps


def run_on_device(x: np.ndarray, Mcat64: np.ndarray, trace: bool = False):
    from concourse.bass_utils import run_bass_kernel_spmd

    if "nc" not in _CACHE:
        _CACHE["nc"] = _build_nc()
    nc = _CACHE["nc"]

    in_maps = _prep_inputs(x, Mcat64)
    res = run_bass_kernel_spmd(nc, in_maps, core_ids=list(range(NCORES)), trace=trace)
    out = np.empty((B, T, D), dtype=np.float32)
    for c in range(NCORES):
        out[c * BSH:(c + 1) * BSH] = res.results[c]["out"].astype(np.float32).reshape(BSH, T, D)
    return out, res


def kernel(x, W, T):
    x = np.asarray(x, dtype=np.float32)
    W = np.asarray(W, dtype=np.float32)
    assert int(T) == 64 and x.shape == (B, D) and W.shape == (D, D)
    Mcat64 = _expm_table(W)
    out, _ = run_on_device(x, Mcat64, trace=False)
    return out
